# revision 1
# baseline (speedup 1.0000x reference)
"""Grouped-query attention + output projection on 8 trn2 NeuronCores.

Sharding: KV group g (and its 4 query heads) -> core g.  Each core computes
its group's attention entirely locally in a transposed layout (scores^T =
[k, q]) so no on-device transposes are needed anywhere:

  mm1:   scoresT[k, q] = kT_tile.T @ qT          (contraction over D=128)
  exp:   ACT Exp with fused 1/sqrt(D) scale, PSUM -> SBUF (fp32r)
  denom: ones[128,1].T @ expT  -> [1, q]         (accumulated over k tiles)
  mm2:   outT[d, q]  = v_tile.T @ expT           (accumulated over k tiles)
  norm:  outT * (ones x 1/denom)                 (broadcast via K=1 matmul)

The per-core attention outputs (concatT slices [512, 2048]) are AllGathered,
then each core computes a 512-column slice of the output projection:
  out[:, mslice] = concat.T @ w_out[mslice, :].T
Host concatenates the 8 column slices.  All matmuls run as float32r
(full-rate fp32 with reduced-precision multiply); walrus requires every
matmul operand to be produced with dtype float32r, so operand tiles and the
DRAM tensors feeding them are declared float32r (bit-identical to fp32).

Host-side prep transposes Q/K and the w_out slice so every operand lands in
SBUF in the exact layout the tensor engine wants.
"""

import sys

import numpy as np

S = 2048
H = 32
G = 8
D = 128
HPG = H // G          # 4 heads per group/core
MODEL = H * D         # 4096
NCORES = 8
MS = MODEL // NCORES  # 512 output columns per core
JS = HPG * D          # 512 concat rows per core
QC = 512              # q-chunk (matmul free dim)
NQC = S // QC         # 4
NKT = S // 128        # 16 k tiles
NJT = MODEL // 128    # 32 j tiles (proj contraction)
SC = 512              # proj s superchunk
NSC = S // SC         # 4

_CACHE = {}


def _build_bass():
    if "/opt/trn_rl_repo" not in sys.path:
        sys.path.insert(0, "/opt/trn_rl_repo")
    import concourse.bacc as bacc
    import concourse.mybir as mybir
    import concourse.tile as tile

    f32 = mybir.dt.float32
    f32r = mybir.dt.float32r
    EXP = mybir.ActivationFunctionType.Exp
    COPY = mybir.ActivationFunctionType.Copy
    scale = float(D) ** -0.5

    nc = bacc.Bacc(None, num_devices=NCORES)
    qT = nc.dram_tensor("qT", [HPG, D, S], f32r, kind="ExternalInput")
    kT = nc.dram_tensor("kT", [D, S], f32r, kind="ExternalInput")
    v = nc.dram_tensor("v", [S, D], f32r, kind="ExternalInput")
    wT = nc.dram_tensor("wT", [MODEL, MS], f32r, kind="ExternalInput")
    ones_d = nc.dram_tensor("ones", [128, 128], f32r, kind="ExternalInput")
    out = nc.dram_tensor("out", [S, MS], f32, kind="ExternalOutput")

    lp = nc.allow_low_precision("fp32r matmul operands")
    lp.__enter__()
    with tile.TileContext(nc) as tc:
        with (
            tc.tile_pool(name="const", bufs=1) as constp,
            tc.tile_pool(name="kv", bufs=1) as kvp,
            tc.tile_pool(name="w", bufs=1) as wp,
            tc.tile_pool(name="qt", bufs=3) as qtp,
            tc.tile_pool(name="expt", bufs=18) as expp,
            tc.tile_pool(name="cc", bufs=3) as ccp,
            tc.tile_pool(name="misc", bufs=4) as miscp,
            tc.tile_pool(name="proj_in", bufs=10) as pip,
            tc.tile_pool(name="out_sb", bufs=3) as outp,
            tc.tile_pool(name="ps_s", bufs=2, space="PSUM") as ps_s,
            tc.tile_pool(name="ps_acc", bufs=4, space="PSUM") as ps_acc,
            tc.tile_pool(name="ps_d", bufs=2, space="PSUM") as ps_d,
            tc.tile_pool(name="dram", bufs=1, space="DRAM") as dramp,
        ):
            # Resident operands
            kT_sb = kvp.tile([128, S], f32r, name="kT_sb")
            nc.sync.dma_start(kT_sb[:], kT[:])
            v_sb = kvp.tile([128, NKT * D], f32r, name="v_sb")
            for t in range(NKT):
                nc.sync.dma_start(
                    v_sb[:, t * D : (t + 1) * D], v[t * 128 : (t + 1) * 128, :]
                )
            wT_sb = wp.tile([128, NJT * MS], f32r, name="wT_sb")
            for a in range(NJT):
                nc.sync.dma_start(
                    wT_sb[:, a * MS : (a + 1) * MS], wT[a * 128 : (a + 1) * 128, :]
                )
            ones_sb = constp.tile([128, 128], f32r, name="ones_sb")
            nc.sync.dma_start(ones_sb[:], ones_d[:])

            cc_in = dramp.tile([JS, S], f32r, name="cc_in")
            cc_out = dramp.tile([MODEL, S], f32r, name="cc_out", addr_space="Shared")

            # Attention (transposed layout, no on-device transposes)
            for c in range(NQC):
                for h in range(HPG):
                    q_sb = qtp.tile([128, QC], f32r, tag="q", name="q_sb")
                    nc.sync.dma_start(q_sb[:], qT[h, :, c * QC : (c + 1) * QC])
                    psum_o = ps_acc.tile([128, QC], f32, tag="acc", name="psum_o")
                    psum_den = ps_d.tile([1, QC], f32, tag="den", name="psum_den")
                    for t in range(NKT):
                        ps = ps_s.tile([128, QC], f32, tag="scores", name="ps")
                        nc.tensor.matmul(
                            ps[:],
                            kT_sb[:, t * 128 : (t + 1) * 128],
                            q_sb[:],
                            start=True,
                            stop=True,
                        )
                        ex = expp.tile([128, QC], f32r, tag="exp", name="ex")
                        nc.scalar.activation(ex[:], ps[:], EXP, scale=scale)
                        nc.tensor.matmul(
                            psum_den[:],
                            ones_sb[:, 0:1],
                            ex[:],
                            start=(t == 0),
                            stop=(t == NKT - 1),
                        )
                        nc.tensor.matmul(
                            psum_o[:],
                            v_sb[:, t * D : (t + 1) * D],
                            ex[:],
                            start=(t == 0),
                            stop=(t == NKT - 1),
                        )
                    recip = miscp.tile([1, QC], f32r, tag="recip", name="recip")
                    nc.vector.reciprocal(recip[:], psum_den[:])
                    ps_b = ps_s.tile([128, QC], f32, tag="scores", name="ps_b")
                    nc.tensor.matmul(
                        ps_b[:],
                        ones_sb[0:1, :],
                        recip[:],
                        start=True,
                        stop=True,
                    )
                    rb_sb = miscp.tile([128, QC], f32, tag="rb", name="rb_sb")
                    nc.scalar.activation(rb_sb[:], ps_b[:], COPY)
                    cc_sb = ccp.tile([128, QC], f32r, tag="cc", name="cc_sb")
                    nc.vector.tensor_mul(cc_sb[:], psum_o[:], rb_sb[:])
                    nc.sync.dma_start(
                        cc_in[h * 128 : (h + 1) * 128, c * QC : (c + 1) * QC],
                        cc_sb[:],
                    )

            nc.gpsimd.collective_compute(
                "AllGather",
                mybir.AluOpType.bypass,
                replica_groups=[list(range(NCORES))],
                ins=[cc_in.opt()],
                outs=[cc_out.opt()],
            )

            # Output projection: out[s, mslice] accumulated over 32 j tiles.
            for sc in range(NSC):
                psums = []
                for si in range(SC // 128):
                    psums.append(
                        ps_acc.tile([128, MS], f32, tag="acc", name="psum_p")
                    )
                for a in range(NJT):
                    lt = pip.tile([128, SC], f32r, tag="pin", name="lt")
                    nc.sync.dma_start(
                        lt[:],
                        cc_out[a * 128 : (a + 1) * 128, sc * SC : (sc + 1) * SC],
                    )
                    for si in range(SC // 128):
                        nc.tensor.matmul(
                            psums[si][:],
                            lt[:, si * 128 : (si + 1) * 128],
                            wT_sb[:, a * MS : (a + 1) * MS],
                            start=(a == 0),
                            stop=(a == NJT - 1),
                        )
                for si in range(SC // 128):
                    o_sb = outp.tile([128, MS], f32, tag="o", name="o_sb")
                    nc.scalar.activation(o_sb[:], psums[si][:], COPY)
                    nc.sync.dma_start(
                        out[(sc * 4 + si) * 128 : (sc * 4 + si + 1) * 128, :],
                        o_sb[:],
                    )
    lp.__exit__(None, None, None)
    nc.finalize()
    return nc


def _get_nc():
    if "nc" not in _CACHE:
        _CACHE["nc"] = _build_bass()
    return _CACHE["nc"]


def _make_in_maps(query, key, value, w_out):
    query = np.asarray(query, dtype=np.float32)
    key = np.asarray(key, dtype=np.float32)
    value = np.asarray(value, dtype=np.float32)
    w_out = np.asarray(w_out, dtype=np.float32)
    ones = np.ones((128, 128), dtype=np.float32)
    in_maps = []
    for g in range(NCORES):
        qTg = np.ascontiguousarray(
            query[:, g * HPG : (g + 1) * HPG, :].transpose(1, 2, 0)
        )  # [HPG, D, S]
        kTg = np.ascontiguousarray(key[:, g, :].T)  # [D, S]
        vg = np.ascontiguousarray(value[:, g, :])  # [S, D]
        wTg = np.ascontiguousarray(w_out[g * MS : (g + 1) * MS, :].T)  # [MODEL, MS]
        in_maps.append({"qT": qTg, "kT": kTg, "v": vg, "wT": wTg, "ones": ones})
    return in_maps


def run_sharded(query, key, value, w_out, trace=False):
    """Run the SPMD kernel; returns (out_full [S, MODEL], BassKernelResults)."""
    if "/opt/trn_rl_repo" not in sys.path:
        sys.path.insert(0, "/opt/trn_rl_repo")
    from concourse.bass_utils import run_bass_kernel_spmd

    nc = _get_nc()
    in_maps = _make_in_maps(query, key, value, w_out)
    res = run_bass_kernel_spmd(nc, in_maps, list(range(NCORES)), trace=trace)
    outs = [np.asarray(res.results[g]["out"]) for g in range(NCORES)]
    full = np.concatenate(outs, axis=1)  # [S, MODEL]
    return full, res


def kernel(query, key, value, mask, w_out, b_out):
    full, _ = run_sharded(query, key, value, w_out, trace=False)
    full = full + np.asarray(b_out, dtype=np.float32)[None, :]
    return full.reshape(S, H, D).astype(np.float32)

